# revision 33
# baseline (speedup 1.0000x reference)
"""Trainium2 Bass kernel: attention-LSTM decoder (nn_Attention_74698071212133).

Sharding: data-parallel over batch across 8 NeuronCores (64 rows each), weights
replicated.  Each core splits its 64 rows into NCHUNK chunks whose attention
phases pipeline against each other; the LSTM/gates phase is fused across chunks
(wider matmuls, N=64) since the recurrence joins there anyway.

Per-core, per step s (batch b=32/chunk, T=64, H=512, C=38):
  h_use = snapshot(hT)                         (DVE)
  hpT   = w_h2h @ h_use       [fused chunks]   (PE)
  per chunk: a = H_projT + bcast_t(hpT)        (DVE, bf16 2x, 2048-wide)
             th = tanh(a)                      (ACT)
             e  = w_score . th -> psum[bt,16]  (PE, lhsT=th slices)
             softmax via PE transpose -> alpha (PE/DVE/ACT, fp32)
             ctxT = enc.T @ alpha-blockdiag    (PE, direct [d, b] layout)
  gatesT = W[ctxT; oh; 1; h_use] [fused]       (PE, K=128-padded onehots)
  lstm elementwise, sigmoid via 0.5*tanh(x/2)+0.5  (ACT+DVE)
  probs = hT.T @ w_genT + b_gen -> DRAM        (PE + DMA)
"""

import sys

sys.path.insert(0, "/opt/trn_rl_repo")

import numpy as np
import ml_dtypes

import concourse.bass as bass
import concourse.mybir as mybir
import concourse.tile as tile
from concourse import bacc
from concourse.bass_utils import run_bass_kernel_spmd

BF = ml_dtypes.bfloat16
F32 = mybir.dt.float32
BF16 = mybir.dt.bfloat16
AF = mybir.ActivationFunctionType
ALU = mybir.AluOpType

# Problem constants
B, T, D, H, C, S = 512, 64, 512, 512, 38, 26
NCORES = 8
BCORE = B // NCORES  # 64
NCHUNK = 2
G4 = 4 * H  # 2048
HK = H // 128  # 4 h-tiles


def _tile128(a):
    """[R, N] with R = r*128 -> [128, r*N] col-block layout (block k = rows 128k..)."""
    r = a.shape[0] // 128
    return np.ascontiguousarray(
        a.reshape(r, 128, a.shape[1]).transpose(1, 0, 2).reshape(128, -1)
    )


def build_nc(steps=S, nchunk=NCHUNK):
    bc = BCORE // nchunk  # batch per chunk
    bt = bc * T  # flattened (b, t) per chunk, b-major
    nbt = bt // 128  # 128-row bt tiles per chunk

    nc = bacc.Bacc()
    dp = nc.declare_dram_parameter
    # Per-core tensors (pre-tiled on host into [128, cols] SBUF images)
    d_enc = dp("enc_sb", [nchunk, 128, nbt * 512], BF16, isOutput=False)
    d_encT = dp("encT_sb", [nchunk, 128, HK * bt], BF16, isOutput=False)
    d_oh = dp("ohT_sb", [128, steps * BCORE], BF16, isOutput=False)
    # Replicated weights
    d_wi2h = dp("w_i2hT", [128, HK * H], BF16, isOutput=False)
    d_wh2h = dp("w_h2hT", [128, HK * H], BF16, isOutput=False)
    d_wsc = dp("w_scoreT", [128, HK], BF16, isOutput=False)
    d_wctx = dp("w_ctxT", [128, HK * G4], BF16, isOutput=False)
    d_whh = dp("w_hhT", [128, HK * G4], BF16, isOutput=False)
    d_woh = dp("w_ohT", [128, G4], BF16, isOutput=False)
    d_wgen = dp("w_genT", [128, HK * C], BF16, isOutput=False)
    d_bgen = dp("b_gen", [1, C], BF16, isOutput=False)
    d_bh2h = dp("b_h2hT", [128, HK], F32, isOutput=False)
    d_idf = dp("id_f32", [128, 128], F32, isOutput=False)
    d_ones = dp("ones_row", [1, BCORE], BF16, isOutput=False)
    d_out = dp("probs", [BCORE, steps, C], F32, isOutput=True)

    with tile.TileContext(nc) as tc:
        with (
            tc.tile_pool(name="consts", bufs=1) as pc,
            tc.tile_pool(name="persist", bufs=1) as pp,
        ):
            # ---- load constants ----
            def cload(dram, shape, dt):
                t_ = pc.tile(list(shape), dt, name=dram.tensor.name + "_sb")
                nc.sync.dma_start(t_[:], dram)
                return t_

            w_i2h = cload(d_wi2h[:], [128, HK * H], BF16)
            w_h2h = cload(d_wh2h[:], [128, HK * H], BF16)
            w_sc = cload(d_wsc[:], [128, HK], BF16)
            w_ctx = cload(d_wctx[:], [128, HK * G4], BF16)
            w_hh = cload(d_whh[:], [128, HK * G4], BF16)
            w_oh = cload(d_woh[:], [128, G4], BF16)
            w_gen = cload(d_wgen[:], [128, HK * C], BF16)
            b_gen = cload(d_bgen[:], [1, C], BF16)
            b_h2h = cload(d_bh2h[:], [128, HK], F32)
            id_f = cload(d_idf[:], [128, 128], F32)
            ones = cload(d_ones[:], [1, BCORE], BF16)
            ohT = cload(d_oh[:], [128, steps * BCORE], BF16)

            # ---- persistent state (fused layout: col-block k is BCORE wide,
            #      [chunk0 bc | chunk1 bc]) ----
            hT = pp.tile([128, HK * BCORE], BF16, tag="hT")
            cT = pp.tile([128, HK * BCORE], F32, tag="cT")
            ctxT = pp.tile([128, HK * BCORE], BF16, tag="ctxT")
            nc.vector.memset(hT[:], 0.0)
            nc.vector.memset(cT[:], 0.0)

            enc_sb, hproj, ad = [], [], []
            for c in range(nchunk):
                e_ = pp.tile([128, nbt * 512], BF16, tag=f"enc{c}")
                for q in range(4):
                    w = nbt * 512 // 4
                    nc.sync.dma_start(
                        e_[:, q * w : (q + 1) * w], d_enc[c, :, q * w : (q + 1) * w]
                    )
                enc_sb.append(e_)
                hproj.append(
                    pp.tile([128, HK * bt], BF16, tag=f"hproj{c}", name=f"hproj{c}")
                )
                a_ = pp.tile([128, bc], BF16, tag=f"ad{c}", name=f"ad{c}")
                nc.vector.memset(a_[:], 0.0)
                ad.append(a_)

            # ---- init: H_projT = w_i2h @ encT + b_h2h ----
            # encT pool is scoped: its recycled addresses give later writers
            # WAW deps on the input DMA queues, but bacc's event-semaphore
            # pass legalizes the wait fan-in.
            with (
                tc.tile_pool(name="encT", bufs=1) as pet,
                tc.tile_pool(name="initps", bufs=4, space="PSUM") as pips,
            ):
                for c in range(nchunk):
                    et = pet.tile([128, HK * bt], BF16, tag=f"encT{c}", name=f"encT{c}")
                    for q in range(4):
                        w = HK * bt // 4
                        nc.sync.dma_start(
                            et[:, q * w : (q + 1) * w],
                            d_encT[c, :, q * w : (q + 1) * w],
                        )
                    for m in range(HK):
                        for n in range(bt // 512):
                            ps = pips.tile([128, 512], F32, tag="initp")
                            for k in range(HK):
                                nc.tensor.matmul(
                                    ps[:],
                                    w_i2h[:, k * H + 128 * m : k * H + 128 * m + 128],
                                    et[:, k * bt + 512 * n : k * bt + 512 * n + 512],
                                    start=(k == 0),
                                    stop=(k == HK - 1),
                                )
                            nc.scalar.activation(
                                hproj[c][:, m * bt + 512 * n : m * bt + 512 * n + 512],
                                ps[:],
                                AF.Identity,
                                bias=b_h2h[:, m : m + 1],
                            )

            # ---- decode steps ----
            with (
                tc.tile_pool(name="work", bufs=6) as pw,
                tc.tile_pool(name="small", bufs=4) as psm,
                tc.tile_pool(name="ps_mix", bufs=2, space="PSUM") as ps_mix,
                tc.tile_pool(name="ps_tr", bufs=1, space="PSUM") as ps_tr,
                tc.tile_pool(name="ps_ctx", bufs=1, space="PSUM") as ps_ctx,
                tc.tile_pool(name="ps_g", bufs=4, space="PSUM") as ps_g,
            ):
                php_holder = [None]
                for s in range(steps):
                    step_body(
                        nc, s, steps, nchunk, bc, bt,
                        pw, psm, ps_mix, ps_tr, ps_ctx, ps_g,
                        enc_sb, hproj, hT, cT, ctxT, ad,
                        w_h2h, w_sc, w_ctx, w_hh, w_oh, w_gen, b_gen,
                        ohT, ones, id_f, d_out, php_holder,
                    )
    if not nc.is_finalized():
        nc.finalize()
    return nc


def step_body(
    nc, s, steps, nchunk, bc, bt,
    pw, psm, ps_mix, ps_tr, ps_ctx, ps_g,
    enc_sb, hproj, hT, cT, ctxT, ad,
    w_h2h, w_sc, w_ctx, w_hh, w_oh, w_gen, b_gen,
    ohT, ones, id_f, d_out, php_holder,
):
    nj = bt // 128
    BW = nchunk * bc  # fused col-block width (BCORE)
    php = php_holder[0]  # hp psum computed during the previous step's LSTM

    # -- gates, h + onehot contributions: emitted first so they are ready as
    #    soon as the previous LSTM tail finishes (reads of hT precede this
    #    step's writes in trace order, so WAR tracking keeps them correct) --
    ohsl = ohT[:, s * BW : (s + 1) * BW]
    pgs = []
    for k in range(HK):
        pg = ps_g.tile([128, 4 * BW], F32, tag="g", name=f"pg{k}", bufs=4)
        pgs.append(pg)
        # exactly ONE start=True per psum bank per step: start marks the whole
        # 2KB zero-region pending, so a second start would wipe other columns'
        # partial sums on their next write
        for gi, gate in enumerate((0, 1, 3, 2)):  # cols = [i, f, o, g]
            m = 4 * gate + k
            col = pg[:, gi * BW : (gi + 1) * BW]
            for kk in range(HK):
                nc.tensor.matmul(
                    col,
                    w_hh[:, kk * G4 + 128 * m : kk * G4 + 128 * m + 128],
                    hT[:, kk * BW : (kk + 1) * BW],
                    start=(gi == 0 and kk == 0),
                    stop=False,
                    skip_group_check=True,
                )
            nc.tensor.matmul(
                col, w_oh[:, 128 * m : 128 * m + 128], ohsl,
                start=False, stop=False, skip_group_check=True,
            )

    # -- attention per chunk (these pipeline against each other) --

    for c in range(nchunk):
        if s > 0:
            # duplicate-x2 hp copies (enable DVE 2x mode on the broadcast add)
            hp2 = psm.tile([128, HK * bc * 2], BF16, tag=f"hp2_{c}")
            hp2v = hp2[:].rearrange("p (m b two) -> p m b two", m=HK, two=2)
            for m in range(HK):
                nc.vector.tensor_copy(
                    hp2v[:, m],
                    php[:, m * BW + c * bc : m * BW + (c + 1) * bc]
                    .unsqueeze(2)
                    .broadcast_to([128, bc, 2]),
                )

        # e scores: per-k matmuls into separate psum blocks (no accumulation
        # groups -> each runs right after its tanh), DVE tree-sum at the end
        pe2 = ps_mix.tile([128, HK * nj], F32, tag="mix", name="pe2")
        for k in range(HK):
            sl = hproj[c][:, k * bt : (k + 1) * bt]
            if s == 0:
                th = pw.tile([128, bt], BF16, tag=f"th{c}", bufs=4)
                nc.scalar.activation(th[:], sl, AF.Tanh)
            else:
                a = pw.tile([128, bt], BF16, tag=f"a{c}", bufs=2)
                nc.vector.tensor_add(
                    a[:].rearrange("p (b t2 two) -> p b t2 two", b=bc, two=2),
                    sl.rearrange("p (b t2 two) -> p b t2 two", b=bc, two=2),
                    hp2v[:, k].unsqueeze(2).broadcast_to([128, bc, T // 2, 2]),
                )
                th = pw.tile([128, bt], BF16, tag=f"th{c}", bufs=4)
                nc.scalar.activation(th[:], a[:], AF.Tanh)
            for j in range(nj):
                nc.tensor.matmul(
                    pe2[:, k * nj + j : k * nj + j + 1],
                    th[:, 128 * j : 128 * j + 128],
                    w_sc[:, k : k + 1],
                    start=True,
                    stop=True,
                )
        e2 = psm.tile([128, nj], F32, tag="e2sb")
        nc.vector.reduce_sum(
            e2[:],
            pe2[:].rearrange("p (k j) -> p j k", k=HK),
            axis=mybir.AxisListType.X,
        )

        # transpose e to [nj, (2b x t)], softmax over t (fp32; no max-sub:
        # |e| <= ||w_score||_1 ~ 20 so fp32 exp is safe)
        ptr = ps_tr.tile([nj, 128], F32, tag="tr")
        nc.tensor.transpose(ptr[:], e2[:], id_f[:])
        ex = psm.tile([nj, 128], F32, tag="ex")
        nc.scalar.activation(ex[:], ptr[:], AF.Exp)
        ssum = psm.tile([nj, 2], F32, tag="ssum")
        nc.vector.reduce_sum(
            ssum[:], ex[:].rearrange("p (b t) -> p b t", b=2), axis=mybir.AxisListType.X
        )
        rinv = psm.tile([nj, 2], F32, tag="rinv")
        nc.vector.reciprocal(rinv[:], ssum[:])
        al = psm.tile([nj, 128], F32, tag="al")
        nc.vector.tensor_mul(
            al[:].rearrange("p (b t) -> p b t", b=2),
            ex[:].rearrange("p (b t) -> p b t", b=2),
            rinv[:].unsqueeze(2).broadcast_to([nj, 2, T]),
        )

        # alpha back to bt-partitions; build block-diag lhsT bands
        pac = ps_tr.tile([128, nj], F32, tag="tr")
        nc.tensor.transpose(pac[:], al[:], id_f[0:nj, 0:nj])
        adv = ad[c][:].rearrange("p (i two) -> p i two", two=2)
        for jj in range(2):
            nc.vector.tensor_copy(
                adv[64 * jj : 64 * jj + 64, :, jj], pac[64 * jj : 64 * jj + 64, :]
            )

        # ctxT[d, b] direct: lhsT = enc tile (d -> partitions), rhs = ad pair
        pctxT = ps_ctx.tile([128, HK * bc], F32, tag="ctxT_ps")
        for m in range(HK):
            for i in range(bc // 2):
                nc.tensor.matmul(
                    pctxT[:, m * bc + 2 * i : m * bc + 2 * i + 2],
                    enc_sb[c][:, 512 * i + 128 * m : 512 * i + 128 * m + 128],
                    ad[c][:, 2 * i : 2 * i + 2],
                    start=True,
                    stop=True,
                )
        nc.vector.tensor_copy(
            ctxT[:].rearrange("p (k b2) -> p k b2", k=HK)[:, :, c * bc : (c + 1) * bc],
            pctxT[:].rearrange("p (k b) -> p k b", k=HK),
        )

    for k in range(HK):
        pg = pgs[k]
        for gi, gate in enumerate((0, 1, 3, 2)):
            m = 4 * gate + k
            col = pg[:, gi * BW : (gi + 1) * BW]
            for cc in range(2):
                for kk in range(HK):
                    nc.tensor.matmul(
                        col[:, cc * (BW // 2) : (cc + 1) * (BW // 2)],
                        w_ctx[:, kk * G4 + 128 * m : kk * G4 + 128 * m + 128],
                        ctxT[
                            :,
                            kk * BW + cc * (BW // 2) : kk * BW + (cc + 1) * (BW // 2),
                        ],
                        start=False,
                        stop=(gi == 3 and cc == 1 and kk == HK - 1),
                        skip_group_check=True,
                    )
        tifo = psm.tile([128, 3 * BW], F32, tag="tifo")
        nc.scalar.activation(tifo[:], pg[:, 0 : 3 * BW], AF.Tanh, scale=0.5)
        tg = psm.tile([128, BW], F32, tag="tg")
        nc.scalar.activation(tg[:], pg[:, 3 * BW : 4 * BW], AF.Tanh)
        sifo = psm.tile([128, 3 * BW], F32, tag="sifo")
        nc.vector.tensor_scalar(sifo[:], tifo[:], 0.5, 0.5, ALU.mult, ALU.add)
        m1 = psm.tile([128, BW], F32, tag="m1")
        nc.vector.tensor_mul(m1[:], sifo[:, BW : 2 * BW], cT[:, k * BW : (k + 1) * BW])
        m2 = psm.tile([128, BW], F32, tag="m2")
        nc.vector.tensor_mul(m2[:], sifo[:, 0:BW], tg[:])
        nc.vector.tensor_add(cT[:, k * BW : (k + 1) * BW], m1[:], m2[:])
        tc_ = psm.tile([128, BW], F32, tag="tc")
        nc.scalar.activation(tc_[:], cT[:, k * BW : (k + 1) * BW], AF.Tanh)
        nc.vector.tensor_mul(
            hT[:, k * BW : (k + 1) * BW], sifo[:, 2 * BW : 3 * BW], tc_[:]
        )
        if s < steps - 1:
            # single accumulation group for the whole php bank (one start)
            if k == 0:
                php_holder[0] = ps_mix.tile(
                    [128, HK * BW], F32, tag="mix", name="php"
                )
            for m in range(HK):
                nc.tensor.matmul(
                    php_holder[0][:, m * BW : (m + 1) * BW],
                    w_h2h[:, k * H + 128 * m : k * H + 128 * m + 128],
                    hT[:, k * BW : (k + 1) * BW],
                    start=(k == 0 and m == 0),
                    stop=(k == HK - 1 and m == HK - 1),
                    skip_group_check=True,
                )

    # -- probs = hT.T @ w_genT + b_gen -> DRAM --
    pp_ = ps_mix.tile([BW, C], F32, tag="mix", name="pp_")
    for k in range(HK):
        nc.tensor.matmul(
            pp_[:],
            hT[:, k * BW : (k + 1) * BW],
            w_gen[:, k * C : (k + 1) * C],
            start=(k == 0),
            stop=False,
            skip_group_check=True,
        )
    nc.tensor.matmul(
        pp_[:], ones[0:1, 0:BW], b_gen[:], start=False, stop=True, skip_group_check=True
    )
    po = psm.tile([BW, C], F32, tag="po")
    nc.vector.tensor_copy(po[:], pp_[:])
    nc.sync.dma_start(d_out[:, s, :], po[:])


# ------------------------- host side -------------------------


def prep_inputs(encoder_output, text, w_i2h, w_h2h, b_h2h, w_score, w_ih, w_hh,
                b_ih, b_hh, w_gen, b_gen, steps=S, nchunk=NCHUNK):
    """Build per-core input maps (numpy only)."""
    bc = BCORE // nchunk
    bt = bc * T
    enc = np.asarray(encoder_output, np.float32)
    text = np.asarray(text)

    wid = {}
    wid["w_i2hT"] = _tile128(np.asarray(w_i2h, np.float32).T.astype(BF))
    wid["w_h2hT"] = _tile128(np.asarray(w_h2h, np.float32).T.astype(BF))
    wid["w_scoreT"] = _tile128(np.asarray(w_score, np.float32).reshape(H, 1).astype(BF))
    wid["w_ctxT"] = _tile128(np.asarray(w_ih, np.float32)[:, :D].T.astype(BF))
    wid["w_hhT"] = _tile128(np.asarray(w_hh, np.float32).T.astype(BF))
    woh = np.zeros((128, G4), BF)  # K padded to 128 so FWL kicks in
    woh[:C] = np.asarray(w_ih, np.float32)[:, D:].T.astype(BF)
    woh[C] = (np.asarray(b_ih, np.float32) + np.asarray(b_hh, np.float32)).astype(BF)
    wid["w_ohT"] = woh
    wid["w_genT"] = _tile128(np.asarray(w_gen, np.float32).T.astype(BF))
    wid["b_gen"] = np.asarray(b_gen, np.float32).reshape(1, C).astype(BF)
    wid["b_h2hT"] = np.ascontiguousarray(
        np.asarray(b_h2h, np.float32).reshape(HK, 128).T
    )
    wid["id_f32"] = np.eye(128, dtype=np.float32)
    wid["ones_row"] = np.ones((1, BCORE), BF)

    in_maps = []
    for core in range(NCORES):
        rows = slice(core * BCORE, (core + 1) * BCORE)
        ec = enc[rows]  # [64, T, D]
        enc_sb = np.zeros((nchunk, 128, (bt // 128) * 512), BF)
        encT_sb = np.zeros((nchunk, 128, HK * bt), BF)
        for c in range(nchunk):
            flat = ec[c * bc : (c + 1) * bc].reshape(bt, D)  # b-major (b,t) rows
            enc_sb[c] = _tile128(flat.astype(BF))
            encT_sb[c] = _tile128(np.ascontiguousarray(flat.T).astype(BF))
        oh = np.zeros((128, steps * BCORE), BF)
        tx = text[rows]  # [64, S]
        for s in range(steps):
            oh[tx[:, s].astype(np.int64), s * BCORE + np.arange(BCORE)] = 1.0
        oh[C] = 1.0
        m = dict(wid)
        m["enc_sb"] = enc_sb
        m["encT_sb"] = encT_sb
        m["ohT_sb"] = oh
        in_maps.append(m)
    return in_maps


_NC_CACHE = {}


def get_nc(steps=S, nchunk=NCHUNK):
    key = (steps, nchunk)
    if key not in _NC_CACHE:
        _NC_CACHE[key] = build_nc(steps, nchunk)
    return _NC_CACHE[key]


def run(inputs, steps=S, nchunk=NCHUNK, trace=False):
    nc = get_nc(steps, nchunk)
    in_maps = prep_inputs(**inputs, steps=steps, nchunk=nchunk)
    res = run_bass_kernel_spmd(nc, in_maps, list(range(NCORES)), trace=trace)
    out = np.concatenate([res.results[i]["probs"] for i in range(NCORES)], axis=0)
    return out.astype(np.float32), res


def kernel(**inputs):
    out, _ = run(inputs)
    return out



# revision 34
# speedup vs baseline: 1.1310x; 1.1310x over previous
"""Trainium2 Bass kernel: attention-LSTM decoder (nn_Attention_74698071212133).

Sharding: data-parallel over batch across 8 NeuronCores (64 rows each), weights
replicated.  Each core runs TWO mostly-independent 32-row recurrence streams
(chunks), phase-staggered half a step apart (tile_wait_until stamps steer the
Tile scheduler) so one stream's serial tail (softmax/ctx/gates/LSTM) hides
under the other stream's attention tanh chain (ScalarE tanh is the hard
per-step floor: B*T*H/8 elems / 128 lanes / 1.2GHz ~ 13.7us/step).

The h-recurrent + onehot gate matmuls and the output projection are fused
across the two streams (N=64, shared weight loads, 16-bit weights); the
ctx-dependent gate half stays per-stream (N=32, fp8 weights) to preserve the
stagger.  Precision: fp16 for all 16-bit tensors; fp8e3 only on the
LDWEIGHTS-bound paths (tanh scores, enc for ctxT, w_ctx), with scales folded
into activation scale parameters; sigmoid via 0.5*(tanh(x/2)+1) with h'=2h so
the 0.5s fold into host-scaled weights; scalar_tensor_tensor fuses (x+1)*y.
"""

import sys

sys.path.insert(0, "/opt/trn_rl_repo")

import numpy as np
import ml_dtypes

import concourse.bass as bass
import concourse.mybir as mybir
import concourse.tile as tile
from concourse import bacc
from concourse.bass_utils import run_bass_kernel_spmd

F16 = np.float16
F8 = ml_dtypes.float8_e3m4
F32 = mybir.dt.float32
FP16 = mybir.dt.float16
FP8 = mybir.dt.float8e3
AF = mybir.ActivationFunctionType
ALU = mybir.AluOpType

# Problem constants
B, T, D, H, C, S = 512, 64, 512, 512, 38, 26
NCORES = 8
BCORE = B // NCORES  # 64
NCHUNK = 2
G4 = 4 * H  # 2048
HK = H // 128  # 4 h-tiles
BC = BCORE // NCHUNK  # 32 batch per stream
BT = BC * T  # 2048 flattened (b, t) per stream
NJ = BT // 128  # 16

# fp8 path toggles
FP8_TH = True   # tanh output + w_score in fp8e3 (e-score LDW fast)
FP8_ENC = False  # enc for ctxT matmul in fp8e3
FP8_W = False    # ctx-gate weights in fp8e3 (x32 scaled)
WS_SC = 32.0 if FP8_TH else 1.0   # w_score host scale, folded out in exp
WG_SC = 32.0 if FP8_W else 1.0    # gate-weight host scale, folded out in ACT
ENC_SC = 4.0 if FP8_ENC else 1.0  # enc host scale, folded out in ctxT copy

TH_DT = FP8 if FP8_TH else FP16
ENC_DT = FP8 if FP8_ENC else FP16
W_DT = FP8 if FP8_W else FP16
TH_NP = F8 if FP8_TH else F16
ENC_NP = F8 if FP8_ENC else F16
W_NP = F8 if FP8_W else F16

# scheduler phase stagger (ms of model time per step)
STEP_MS = 0.021

# gate source order producing psum col layout [i | f | o | g]
GATE_ORDER = (0, 1, 3, 2)

# per-stream psum scratch-bank column layout (f32 cols of a [128, 512] tile)
PE20, PE21 = 0, NJ                    # e-scores [128, 16]
PTR0, PTR1 = 16, 144                  # e transposed [16, 128]
PAC0, PAC1 = 144, 160                 # alpha transposed back [128, 16]
PCTX0, PCTX1 = 160, 288               # ctxT [128, 128]
PP0, PP1 = 288, 326                   # probs [64, 38] (stream-0 bank only)
PHP0, PHP1 = 326, 454                 # hp for next step [128, 128]


def _tile128(a, dt):
    """[R, N] with R = r*128 -> [128, r*N] col-block layout (block k = rows 128k..)."""
    r = a.shape[0] // 128
    return np.ascontiguousarray(
        a.reshape(r, 128, a.shape[1]).transpose(1, 0, 2).reshape(128, -1)
    ).astype(dt)


class Ctx:
    """Per-build handles."""


def build_nc(steps=S):
    nc = bacc.Bacc()
    dp = nc.declare_dram_parameter
    x = Ctx()
    x.nc = nc
    x.steps = steps

    d_enc = dp("enc_sb", [NCHUNK, 128, NJ * 512], ENC_DT, isOutput=False)
    d_encT = dp("encT_sb", [NCHUNK, 128, HK * BT], FP16, isOutput=False)
    d_oh = dp("ohT_sb", [128, steps * BCORE], FP16, isOutput=False)
    d_wi2h = dp("w_i2hT", [128, HK * H], FP16, isOutput=False)
    d_wh2h = dp("w_h2hT", [128, HK * H], FP16, isOutput=False)
    d_wsc = dp("w_scoreT", [128, HK], TH_DT, isOutput=False)
    d_wctx = dp("w_ctxT", [128, HK * G4], W_DT, isOutput=False)
    d_whh = dp("w_hhT", [128, HK * G4], FP16, isOutput=False)
    d_woh = dp("w_ohT", [128, G4], FP16, isOutput=False)
    d_wgen = dp("w_genT", [128, HK * C], FP16, isOutput=False)
    d_bgen = dp("b_gen", [1, C], FP16, isOutput=False)
    d_bh2h = dp("b_h2hT", [128, HK], F32, isOutput=False)
    d_idf = dp("id_f", [128, 128], F32, isOutput=False)
    d_ones = dp("ones_row", [1, BCORE], FP16, isOutput=False)
    d_out = dp("probs", [BCORE, steps, C], F32, isOutput=True)

    with tile.TileContext(nc) as tc:
        with (
            tc.tile_pool(name="consts", bufs=1) as pc,
            tc.tile_pool(name="persist", bufs=1) as pp,
        ):
            def cload(dram, shape, dt):
                t_ = pc.tile(list(shape), dt, name=dram.tensor.name + "_sb")
                nc.sync.dma_start(t_[:], dram)
                return t_

            x.w_h2h = cload(d_wh2h[:], [128, HK * H], FP16)
            x.w_sc = cload(d_wsc[:], [128, HK], TH_DT)
            x.w_ctx = cload(d_wctx[:], [128, HK * G4], W_DT)
            x.w_hh = cload(d_whh[:], [128, HK * G4], FP16)
            x.w_oh = cload(d_woh[:], [128, G4], FP16)
            x.w_gen = cload(d_wgen[:], [128, HK * C], FP16)
            x.b_gen = cload(d_bgen[:], [1, C], FP16)
            x.id_f = cload(d_idf[:], [128, 128], F32)
            x.ones = cload(d_ones[:], [1, BCORE], FP16)
            x.ohT = cload(d_oh[:], [128, steps * BCORE], FP16)
            w_i2h = cload(d_wi2h[:], [128, HK * H], FP16)
            b_h2h = cload(d_bh2h[:], [128, HK], F32)

            # ---- persistent state: hT fused across streams, rest per-stream ----
            x.hTf = pp.tile([128, HK * BCORE], FP16, tag="hTf", name="hTf")
            nc.vector.memset(x.hTf[:], 0.0)
            x.cT, x.ctxT, x.ad, x.enc_sb, x.hproj = [], [], [], [], []
            for c in range(NCHUNK):
                c_ = pp.tile([128, HK * BC], F32, tag=f"cT{c}", name=f"cT{c}")
                ct = pp.tile([128, HK * BC], FP16, tag=f"ctxT{c}", name=f"ctxT{c}")
                a_ = pp.tile([128, 2 * NJ], FP16, tag=f"ad{c}", name=f"ad{c}")
                nc.vector.memset(a_[:], 0.0)
                x.cT.append(c_)
                x.ctxT.append(ct)
                x.ad.append(a_)
                e_ = pp.tile([128, NJ * 512], ENC_DT, tag=f"enc{c}", name=f"enc{c}")
                for q in range(4):
                    w = NJ * 512 // 4
                    nc.sync.dma_start(
                        e_[:, q * w : (q + 1) * w], d_enc[c, :, q * w : (q + 1) * w]
                    )
                x.enc_sb.append(e_)
                x.hproj.append(
                    pp.tile([128, HK * BT], FP16, tag=f"hproj{c}", name=f"hproj{c}")
                )

            # ---- init: H_projT = w_i2h @ encT + b_h2h ----
            with (
                tc.tile_pool(name="encT", bufs=1) as pet,
                tc.tile_pool(name="initps", bufs=4, space="PSUM") as pips,
            ):
                for c in range(NCHUNK):
                    et = pet.tile([128, HK * BT], FP16, tag=f"encT{c}", name=f"encT{c}")
                    for q in range(4):
                        w = HK * BT // 4
                        nc.sync.dma_start(
                            et[:, q * w : (q + 1) * w],
                            d_encT[c, :, q * w : (q + 1) * w],
                        )
                    for m in range(HK):
                        for n in range(BT // 512):
                            ps = pips.tile([128, 512], F32, tag="initp")
                            for k in range(HK):
                                nc.tensor.matmul(
                                    ps[:],
                                    w_i2h[:, k * H + 128 * m : k * H + 128 * m + 128],
                                    et[:, k * BT + 512 * n : k * BT + 512 * n + 512],
                                    start=(k == 0),
                                    stop=(k == HK - 1),
                                )
                            nc.scalar.activation(
                                x.hproj[c][:, m * BT + 512 * n : m * BT + 512 * n + 512],
                                ps[:],
                                AF.Identity,
                                bias=b_h2h[:, m : m + 1],
                            )

            # ---- decode: phase-staggered two-stream pipeline ----
            with (
                tc.tile_pool(name="work", bufs=1) as pw,
                tc.tile_pool(name="small", bufs=1) as psm,
                tc.tile_pool(name="ps", bufs=1, space="PSUM") as ps,
            ):
                x.pw, x.psm, x.ps = pw, psm, ps
                x.d_out = d_out
                x.pg = None
                x.sc = [
                    ps.tile([128, 512], F32, tag=f"sc{c}", name=f"sc{c}", bufs=1)
                    for c in range(NCHUNK)
                ]
                x.th = [[None, None], [None, None]]
                x.a = [[None, None], [None, None]]
                x.hp2 = [None, None]
                x.e2 = [None, None]
                x.ex = [None, None]
                x.tifo = [None, None]

                F = tc.no_sync_barrier
                x.F = F
                # prologue: stream-0 step-0 attention + pre-softmax
                gates_hof(x, 0)
                for k in range(HK):
                    tanh_k(x, 0, 0, k)
                    escore_k(x, 0, 0, k)
                    F()
                presoftmax(x, 0)
                F()
                # steady state: half-periods; A = tail stream (step s),
                # B = other stream running its attention tanh, interleaved
                # so B's tanh quarters fill A's chain gaps on ACT.
                for hs in range(2 * steps):
                    A = hs % 2
                    s = hs // 2
                    Bc = 1 - A
                    sB = s + A  # B's attention step
                    runB = sB < steps
                    exp_a(x, A)
                    if runB and sB > 0:
                        add_k(x, Bc, sB, 1)
                    F()
                    if runB:
                        tanh_k(x, Bc, sB, 0)
                    alpha_chain(x, A, s)
                    F()
                    if runB and sB > 0:
                        add_k(x, Bc, sB, 2)
                    ctx_mm(x, A, s)
                    if runB:
                        escore_k(x, Bc, sB, 0)
                    F()
                    if runB:
                        tanh_k(x, Bc, sB, 1)
                    ctx_gates(x, A, s)
                    if runB and sB > 0:
                        add_k(x, Bc, sB, 3)
                    F()
                    if runB:
                        tanh_k(x, Bc, sB, 2)
                        escore_k(x, Bc, sB, 1)
                    F()
                    lstm1(x, A, s)
                    F()
                    lstm2(x, A, s)
                    if runB:
                        escore_k(x, Bc, sB, 2)
                    F()
                    if A == 1 and s + 1 < steps:
                        gates_hof(x, s + 1)
                        F()
                    if runB:
                        tanh_k(x, Bc, sB, 3)
                    if s < steps - 1:
                        php_mm(x, A, s)
                    if A == 1:
                        probs_f(x, s)
                    F()
                    if s < steps - 1:
                        add_k(x, A, s + 1, 0)  # incl hp2
                    if runB:
                        escore_k(x, Bc, sB, 3)
                        presoftmax(x, Bc)
                    F()
    if not nc.is_finalized():
        nc.finalize()
    return nc


def gates_hof(x, s):
    """Fused h-recurrent + onehot gate matmuls for BOTH streams (N=64).
    pg col layout [gi(4), k(4), b(64)] = [i | f | o | g], banks A=(i,f) B=(o,g)."""
    nc = x.nc
    pg = x.ps.tile([128, 1024], F32, tag="pg", name=f"pg{s}", bufs=2)
    x.pg = pg
    ohsl = x.ohT[:, s * BCORE : (s + 1) * BCORE]
    started = set()
    for gi, g in enumerate(GATE_ORDER):
        for k in range(HK):
            m = 4 * g + k
            col = pg[:, gi * 256 + k * BCORE : gi * 256 + (k + 1) * BCORE]
            bank = gi // 2
            for kk in range(HK):
                nc.tensor.matmul(
                    col,
                    x.w_hh[:, kk * G4 + 128 * m : kk * G4 + 128 * m + 128],
                    x.hTf[:, kk * BCORE : (kk + 1) * BCORE],
                    start=(bank not in started),
                    stop=False,
                    skip_group_check=True,
                )
                started.add(bank)
            nc.tensor.matmul(
                col, x.w_oh[:, 128 * m : 128 * m + 128], ohsl,
                start=False, stop=False, skip_group_check=True,
            )


def probs_f(x, s):
    """Fused output projection for both streams: [64, C] psum -> DRAM."""
    nc = x.nc
    pp_ = x.sc[1][0:BCORE, PP0:PP1]
    for k in range(HK):
        nc.tensor.matmul(
            pp_,
            x.hTf[:, k * BCORE : (k + 1) * BCORE],
            x.w_gen[:, k * C : (k + 1) * C],
            start=(k == 0),
            stop=False,
            skip_group_check=True,
        )
    nc.tensor.matmul(
        pp_, x.ones[0:1, :], x.b_gen[:],
        start=False, stop=True, skip_group_check=True,
    )
    po = x.psm.tile([BCORE, C], F32, tag="po", name="po", bufs=2)
    nc.vector.tensor_copy(po[:], pp_)
    nc.sync.dma_start(x.d_out[:, s, :], po[:])


def add_k(x, c, s, k):
    """One per-k attention add (DVE); k==0 also makes the hp duplicate-x2."""
    nc = x.nc
    kp, kk = k // 2, k % 2
    if k == 0:
        hp2 = x.psm.tile([128, HK * BC * 2], FP16, tag=f"hp2_{c}", name=f"hp2_{c}",
                         bufs=2)
        x.hp2[c] = hp2
        nc.vector.tensor_copy(
            hp2[:].rearrange("p (k b two) -> p k b two", k=HK, two=2),
            x.sc[c][:, PHP0:PHP1]
            .rearrange("p (k b) -> p k b", k=HK)
            .unsqueeze(3)
            .broadcast_to([128, HK, BC, 2]),
        )
    if kk == 0:
        x.a[c][kp] = x.pw.tile([128, 2 * BT], FP16, tag=f"a{c}",
                               name=f"a{c}_{kp}", bufs=2)
    hp2v = x.hp2[c][:].rearrange("p (k b two) -> p k b two", k=HK, two=2)
    nc.vector.tensor_add(
        x.a[c][kp][:, kk * BT : (kk + 1) * BT].rearrange(
            "p (b t2 two) -> p b t2 two", b=BC, two=2
        ),
        x.hproj[c][:, k * BT : (k + 1) * BT].rearrange(
            "p (b t2 two) -> p b t2 two", b=BC, two=2
        ),
        hp2v[:, k].unsqueeze(2).broadcast_to([128, BC, T // 2, 2]),
    )


def tanh_k(x, c, s, k):
    """One k-tile of the attention tanh (ACT, [128, 2048])."""
    nc = x.nc
    kp, kk = k // 2, k % 2
    if kk == 0:
        x.th[c][kp] = x.pw.tile([128, 2 * BT], TH_DT, tag=f"th{c}",
                                name=f"th{c}_{kp}", bufs=2)
    th = x.th[c][kp]
    if s == 0:
        src_ = x.hproj[c][:, k * BT : (k + 1) * BT]
    else:
        src_ = x.a[c][kp][:, kk * BT : (kk + 1) * BT]
    nc.scalar.activation(th[:, kk * BT : (kk + 1) * BT], src_, AF.Tanh)


def escore_k(x, c, s, k):
    """e-score contribution of one k-tile (PE): pe2[:, j] += th_k.T @ w_sc[k]."""
    nc = x.nc
    kp, kk = k // 2, k % 2
    th = x.th[c][kp]
    pe2 = x.sc[c][:, PE20:PE21]
    for j in range(NJ):
        nc.tensor.matmul(
            pe2[:, j : j + 1],
            th[:, kk * BT + 128 * j : kk * BT + 128 * j + 128],
            x.w_sc[:, k : k + 1],
            start=(k == 0 and j == 0),
            stop=(k == HK - 1 and j == NJ - 1),
            skip_group_check=True,
        )


def presoftmax(x, c):
    """e2 copy + transpose, hoisted to overlap the preceding tanh tail."""
    nc = x.nc
    e2 = x.psm.tile([128, NJ], F32, tag=f"e2_{c}", name=f"e2_{c}", bufs=2)
    x.e2[c] = e2
    nc.vector.tensor_copy(e2[:], x.sc[c][:, PE20:PE21])
    nc.tensor.transpose(x.sc[c][0:NJ, PTR0:PTR1], e2[:], x.id_f[:])


def exp_a(x, c):
    nc = x.nc
    ex = x.psm.tile([NJ, 128], FP16, tag=f"ex_{c}", name=f"ex_{c}", bufs=2)
    x.ex[c] = ex
    nc.scalar.activation(ex[:], x.sc[c][0:NJ, PTR0:PTR1], AF.Exp, scale=1.0 / WS_SC)


def alpha_chain(x, c, s):
    """alpha = ex/sum -> block-diag bands (DVE+PE)."""
    nc = x.nc
    ex = x.ex[c]
    ssum = x.psm.tile([NJ, 2], F32, tag=f"ssum_{c}", name=f"ssum_{c}", bufs=2)
    nc.vector.reduce_sum(
        ssum[:], ex[:].rearrange("p (b t) -> p b t", b=2), axis=mybir.AxisListType.X
    )
    rinv = x.psm.tile([NJ, 2], F32, tag=f"rinv_{c}", name=f"rinv_{c}", bufs=2)
    nc.vector.reciprocal(rinv[:], ssum[:])
    al = x.psm.tile([NJ, 128], F32, tag=f"al_{c}", name=f"al_{c}", bufs=2)
    nc.vector.tensor_mul(
        al[:].rearrange("p (b t) -> p b t", b=2),
        ex[:].rearrange("p (b t) -> p b t", b=2),
        rinv[:].unsqueeze(2).broadcast_to([NJ, 2, T]),
    )
    pac = x.sc[c][:, PAC0:PAC1]
    nc.tensor.transpose(pac, al[:], x.id_f[0:NJ, 0:NJ])
    adv = x.ad[c][:].rearrange("p (i two) -> p i two", two=2)
    for jj in range(2):
        nc.vector.tensor_copy(
            adv[64 * jj : 64 * jj + 64, :, jj], pac[64 * jj : 64 * jj + 64, :]
        )


def ctx_mm(x, c, s):
    """ctxT[d, b]: lhsT = enc tile (d -> partitions), rhs = ad pair (PE);
    per-m psum->SBUF copies interleaved so ctx_gates can chase per-kk."""
    nc = x.nc
    pctxT = x.sc[c][:, PCTX0:PCTX1]
    for m in range(HK):
        for i in range(BC // 2):
            nc.tensor.matmul(
                pctxT[:, m * BC + 2 * i : m * BC + 2 * i + 2],
                x.enc_sb[c][:, 512 * i + 128 * m : 512 * i + 128 * m + 128],
                x.ad[c][:, 2 * i : 2 * i + 2],
                start=True,
                stop=True,
            )
        nc.vector.tensor_scalar_mul(
            x.ctxT[c][:, m * BC : (m + 1) * BC],
            pctxT[:, m * BC : (m + 1) * BC],
            1.0 / ENC_SC,
        )


def ctx_gates(x, c, s):
    """ctx half of the gates (PE, fp8)."""
    nc = x.nc
    pg = x.pg
    for kk in range(HK):
        for gi, g in enumerate(GATE_ORDER):
            for k in range(HK):
                m = 4 * g + k
                col = pg[:, gi * 256 + k * BCORE + c * BC : gi * 256 + k * BCORE + (c + 1) * BC]
                nc.tensor.matmul(
                    col,
                    x.w_ctx[:, kk * G4 + 128 * m : kk * G4 + 128 * m + 128],
                    x.ctxT[c][:, kk * BC : (kk + 1) * BC],
                    start=False,
                    stop=(c == 1 and gi % 2 == 1 and k == HK - 1 and kk == HK - 1),
                    skip_group_check=True,
                )


def lstm1(x, c, s):
    """ifo + g tanh (ACT), cell-update muls (DVE)."""
    nc = x.nc
    pgv = x.pg[:].rearrange("p (gi k b) -> p gi k b", gi=4, k=HK)
    tifo = x.psm.tile([128, 3 * HK * BC], FP16, tag=f"tifo_{c}", name=f"tifo_{c}",
                      bufs=2)
    x.tifo[c] = tifo
    nc.scalar.activation(
        tifo[:].rearrange("p (gi k b) -> p gi k b", gi=3, k=HK),
        pgv[:, 0:3, :, c * BC : (c + 1) * BC],
        AF.Tanh, scale=0.5 / WG_SC,
    )
    tg = x.psm.tile([128, HK * BC], FP16, tag=f"tg_{c}", name=f"tg_{c}", bufs=2)
    nc.scalar.activation(
        tg[:].rearrange("p (k b) -> p k b", k=HK),
        pgv[:, 3, :, c * BC : (c + 1) * BC],
        AF.Tanh, scale=1.0 / WG_SC,
    )
    # C' = 2c: C'_new = 0.5*(tanh_f+1)*C'_old + (tanh_i+1)*tanh_g
    m2 = x.psm.tile([128, HK * BC], F32, tag=f"m2_{c}", name=f"m2_{c}", bufs=2)
    nc.vector.scalar_tensor_tensor(
        m2[:], tifo[:, 0 : HK * BC], 1.0, tg[:], ALU.add, ALU.mult
    )
    if s == 0:
        nc.vector.tensor_copy(x.cT[c][:], m2[:])
    else:
        m1 = x.psm.tile([128, HK * BC], F32, tag=f"m1_{c}", name=f"m1_{c}", bufs=2)
        nc.vector.scalar_tensor_tensor(
            m1[:], tifo[:, HK * BC : 2 * HK * BC], 1.0, x.cT[c][:], ALU.add, ALU.mult
        )
        nc.vector.scalar_tensor_tensor(
            x.cT[c][:], m1[:], 0.5, m2[:], ALU.mult, ALU.add
        )


def lstm2(x, c, s):
    """tanh(c) + h' write (ACT+DVE)."""
    nc = x.nc
    tifo = x.tifo[c]
    tc_ = x.psm.tile([128, HK * BC], FP16, tag=f"tc_{c}", name=f"tc_{c}", bufs=2)
    nc.scalar.activation(tc_[:], x.cT[c][:], AF.Tanh, scale=0.5)
    nc.vector.scalar_tensor_tensor(
        x.hTf[:].rearrange("p (k b) -> p k b", k=HK)[:, :, c * BC : (c + 1) * BC],
        tifo[:].rearrange("p (gi k b) -> p gi k b", gi=3, k=HK)[:, 2],
        1.0,
        tc_[:].rearrange("p (k b) -> p k b", k=HK),
        ALU.add, ALU.mult,
    )


def php_mm(x, c, s):
    """php = w_h2h' @ H' for stream c (PE)."""
    nc = x.nc
    php = x.sc[c][:, PHP0:PHP1]
    for m in range(HK):
        for k in range(HK):
            nc.tensor.matmul(
                php[:, m * BC : (m + 1) * BC],
                x.w_h2h[:, k * H + 128 * m : k * H + 128 * m + 128],
                x.hTf[:, k * BCORE + c * BC : k * BCORE + (c + 1) * BC],
                start=(m == 0 and k == 0),
                stop=(m == HK - 1 and k == HK - 1),
                skip_group_check=True,
            )


# ------------------------- host side -------------------------


def prep_inputs(encoder_output, text, w_i2h, w_h2h, b_h2h, w_score, w_ih, w_hh,
                b_ih, b_hh, w_gen, b_gen, steps=S):
    """Build per-core input maps (numpy only)."""
    enc = np.asarray(encoder_output, np.float32)
    text = np.asarray(text)

    wid = {}
    wid["w_i2hT"] = _tile128(np.asarray(w_i2h, np.float32).T, F16)
    # w_h2h' = 0.5*w_h2h: php(H') = w_h2h @ h exactly
    wid["w_h2hT"] = _tile128(0.5 * np.asarray(w_h2h, np.float32).T, F16)
    wid["w_scoreT"] = _tile128(
        WS_SC * np.asarray(w_score, np.float32).reshape(H, 1), TH_NP
    )
    # gate weights: ctx part xWG, h part xWG/2 (h'=2h), oh part + bias xWG
    wid["w_ctxT"] = _tile128(WG_SC * np.asarray(w_ih, np.float32)[:, :D].T, W_NP)
    wid["w_hhT"] = _tile128(0.5 * WG_SC * np.asarray(w_hh, np.float32).T, F16)
    woh = np.zeros((128, G4), np.float32)
    woh[:C] = WG_SC * np.asarray(w_ih, np.float32)[:, D:].T
    woh[C] = WG_SC * (np.asarray(b_ih, np.float32) + np.asarray(b_hh, np.float32))
    wid["w_ohT"] = woh.astype(F16)
    wid["w_genT"] = _tile128(0.5 * np.asarray(w_gen, np.float32).T, F16)
    wid["b_gen"] = np.asarray(b_gen, np.float32).reshape(1, C).astype(F16)
    wid["b_h2hT"] = np.ascontiguousarray(
        np.asarray(b_h2h, np.float32).reshape(HK, 128).T
    )
    wid["id_f"] = np.eye(128, dtype=np.float32)
    wid["ones_row"] = np.ones((1, BCORE), F16)

    in_maps = []
    for core in range(NCORES):
        rows = slice(core * BCORE, (core + 1) * BCORE)
        ec = enc[rows]  # [64, T, D]
        enc_sb = np.zeros((NCHUNK, 128, NJ * 512), ENC_NP)
        encT_sb = np.zeros((NCHUNK, 128, HK * BT), F16)
        for c in range(NCHUNK):
            flat = ec[c * BC : (c + 1) * BC].reshape(BT, D)  # b-major (b,t) rows
            enc_sb[c] = _tile128(np.clip(ENC_SC * flat, -15.5, 15.5), ENC_NP)
            encT_sb[c] = _tile128(np.ascontiguousarray(flat.T), F16)
        oh = np.zeros((128, steps * BCORE), F16)
        tx = text[rows]  # [64, S]
        for s in range(steps):
            oh[tx[:, s].astype(np.int64), s * BCORE + np.arange(BCORE)] = 1.0
        oh[C] = 1.0
        m = dict(wid)
        m["enc_sb"] = enc_sb
        m["encT_sb"] = encT_sb
        m["ohT_sb"] = oh
        in_maps.append(m)
    return in_maps


_NC_CACHE = {}


def get_nc(steps=S):
    if steps not in _NC_CACHE:
        _NC_CACHE[steps] = build_nc(steps)
    return _NC_CACHE[steps]


def run(inputs, steps=S, trace=False):
    nc = get_nc(steps)
    in_maps = prep_inputs(**inputs, steps=steps)
    res = run_bass_kernel_spmd(nc, in_maps, list(range(NCORES)), trace=trace)
    out = np.concatenate([res.results[i]["probs"] for i in range(NCORES)], axis=0)
    return out.astype(np.float32), res


def kernel(**inputs):
    out, _ = run(inputs)
    return out
